# revision 40
# baseline (speedup 1.0000x reference)
"""Ternary-weight linear layer on 8 Trainium2 NeuronCores.

Problem: y = x @ ternarize(W).T + b
  x [8192, 4096] fp32, W [4096, 4096] fp32, b [4096] fp32.
  ternarize(w) = round(clamp(w, -1, 1))  (round-half-even, forward value).

Strategy (data-parallel over tokens, replicated weights, fp8 DoubleRow):
  - Each of the 8 cores gets 1024 tokens. Host passes x and W transposed
    (layout-only prep) so the contraction dim i lands on SBUF partitions:
        xT  [4096 i, 1024 t]  (per-core slice)
        wT  [4096 i, 4096 o]  (replicated)
  - W is ternarized exactly on device with two DVE tensor_scalar ops:
    clamp via min/max (f32), then round-half-even via +C/-C (C = 1.5*2^23)
    writing float8e4 directly ({-1, 0, 1} are exact in fp8).
  - x is split on device into x_hi = fp8(x) and x_lo = fp8(x - x_hi).
    Two fp8 matmul passes accumulate into the same PSUM region; combined
    x quantization error is ~2^-8 relative (well under tolerance).
  - Matmuls use MatmulPerfMode.DoubleRow: lhsT [128, 2, 128] fp8 and
    rhs [128, 2, 256] fp8 contract TWO 128-deep k-slabs per instruction
    at 0.5 cycles per output column - 2x the bf16/f32r rate.
  - Pipeline: x quarters and the first W chunks are interleaved on one
    DMA queue (and in DVE issue order) so the PE starts early; each
    (chunk, o-block, token-quarter) cell accumulates in its own PSUM
    bank (8 banks rotating) and is evicted as soon as it stops.
  - Bias is added during PSUM->SBUF eviction on the scalar engine; the
    output is written as bf16 (halves the y DMA traffic; |rel err| ~2^-9).
  - Per-core output yT [4096 o, 1024 t] bf16; host casts + transposes.
"""

import contextlib

import numpy as np

N_CORES = 8
TOKENS = 8192
IN_F = 4096
OUT_F = 4096
T_CORE = TOKENS // N_CORES       # 1024 tokens per core
P = 128                          # partitions
KB = IN_F // P                   # 32 contraction slabs of 128
NQ = 4                           # x token quarters
TQ = T_CORE // NQ                # 256 tokens per quarter / per matmul
O_CHUNK = 128                    # o columns per W chunk
N_CHUNKS = OUT_F // O_CHUNK      # 16
C_ROUND = 12582912.0             # 1.5 * 2^23; (v+C)-C == round-half-even(v)

_built = None


def _build(reps=1):
    import concourse.bacc as bacc
    import concourse.mybir as mybir
    import concourse.tile as tile

    dt = mybir.dt
    DR = mybir.MatmulPerfMode.DoubleRow

    nc = bacc.Bacc("TRN2", target_bir_lowering=False, debug=False)
    xT_d = nc.dram_tensor("xT", [IN_F, T_CORE], dt.float32, kind="ExternalInput").ap()
    wT_d = nc.dram_tensor("wT", [IN_F, OUT_F], dt.float32, kind="ExternalInput").ap()
    biasT_d = nc.dram_tensor("biasT", [P, OUT_F // P], dt.float32,
                             kind="ExternalInput").ap()
    yT_d = nc.dram_tensor("yT", [OUT_F, T_CORE], dt.bfloat16,
                          kind="ExternalOutput").ap()

    xT_r = xT_d.rearrange("(kb p) t -> p kb t", p=P)     # [128, 32, 1024]
    wT_r = wT_d.rearrange("(kb p) o -> p kb o", p=P)     # [128, 32, 4096]

    with tile.TileContext(nc) as tc:
        with tc.tile_pool(name="xq", bufs=1) as xq, \
             tc.tile_pool(name="xf", bufs=4) as xf, \
             tc.tile_pool(name="wf", bufs=4) as wfp, \
             tc.tile_pool(name="wq", bufs=8) as wqp, \
             tc.tile_pool(name="op", bufs=8) as op, \
             tc.tile_pool(name="cn", bufs=1) as cn, \
             tc.tile_pool(name="ps", bufs=8, space="PSUM") as ps:

            biasT = cn.tile([P, OUT_F // P], dt.float32, name="biasT_s")
            nc.sync.dma_start(out=biasT[:], in_=biasT_d[:])

            # All streamed tensors are split into k-slab halves (16 slabs
            # each) as SEPARATE tiles: dependencies are tile-granular, so
            # halving the tiles halves every pipeline latency (DMA, convert,
            # first-matmul).  Matmuls s=0..7 read half 0, s=8..15 half 1.
            KH = KB // 2                       # 16 slabs per half
            xhi = [[None, None] for _ in range(NQ)]
            xlo = [[None, None] for _ in range(NQ)]

            def load_quarter_hi(q):
                """DMA one x token-quarter (two slab-halves) and convert
                the fp8 hi parts."""
                stages = []
                for sh in range(2):
                    stage = xf.tile([P, KH, TQ], dt.float32, tag="xf",
                                    name=f"xf{q}_{sh}")
                    nc.sync.dma_start(
                        out=stage[:],
                        in_=xT_r[:, sh * KH:(sh + 1) * KH,
                                 q * TQ:(q + 1) * TQ])
                    hi = xq.tile([P, KH, TQ], dt.float8e4, tag=f"xh{q}{sh}",
                                 name=f"xh{q}_{sh}")
                    # pure cast on the (prologue-idle) scalar engine keeps
                    # the DVE free for lo-residuals and W ternarize
                    nc.scalar.activation(
                        hi[:], stage[:],
                        mybir.ActivationFunctionType.Identity, scale=1.0)
                    xhi[q][sh] = hi
                    stages.append(stage)
                return stages

            def load_quarter_lo(q, stages):
                """Convert the fp8 lo (residual) parts of a token-quarter."""
                for sh in range(2):
                    lo = xq.tile([P, KH, TQ], dt.float8e4, tag=f"xl{q}{sh}",
                                 name=f"xl{q}_{sh}")
                    nc.vector.tensor_sub(lo[:], stages[sh][:],
                                         xhi[q][sh][:])
                    xlo[q][sh] = lo

            def load_chunk_half(ch, h):
                """DMA one W o-block (128 outs, two slab-halves) and
                ternarize into fp8 tiles."""
                lo_o = ch * O_CHUNK + h * P
                out = []
                for sh in range(2):
                    wf = wfp.tile([P, KH, P], dt.float32, tag="wf",
                                  name=f"wf{ch}_{h}_{sh}")
                    nc.sync.dma_start(
                        out=wf[:],
                        in_=wT_r[:, sh * KH:(sh + 1) * KH, lo_o:lo_o + P])
                    nc.vector.tensor_scalar(wf[:], wf[:], 1.0, -1.0,
                                            mybir.AluOpType.min,
                                            mybir.AluOpType.max)
                    wq = wqp.tile([P, KH, P], dt.float8e4, tag="wq",
                                  name=f"wq{ch}_{h}_{sh}")
                    nc.vector.tensor_scalar(wq[:], wf[:], C_ROUND, C_ROUND,
                                            mybir.AluOpType.add,
                                            mybir.AluOpType.subtract)
                    out.append(wq)
                return out

            def cell(wq, ch, ob, tb):
                """One (chunk, o-block, token-quarter) accumulation: 32
                DoubleRow matmuls into a private PSUM bank, then evict with
                bias into bf16 and DMA the [128, 256] piece out."""
                o_abs = ch * O_CHUNK + ob * P
                psum = ps.tile([P, 512], dt.float32, tag="ps",
                               name=f"ps_{ch}_{ob}_{tb}")
                # hi pass first, then lo: the lo conversion (DVE) hides
                # under the hi-pass matmuls
                for pi, xsrc in enumerate((xhi[tb], xlo[tb])):
                    for s in range(KB // 2):
                        sh, sl = divmod(s, KH // 2)
                        nc.tensor.matmul(
                            psum[:, :TQ],
                            wq[sh][:, 2 * sl:2 * sl + 2, :],
                            xsrc[sh][:, 2 * sl:2 * sl + 2, :],
                            start=(s == 0 and pi == 0),
                            stop=(s == KB // 2 - 1 and pi == 1),
                            perf_mode=DR)
                stage = op.tile([P, TQ], dt.bfloat16, tag="out",
                                name=f"out_{ch}_{ob}_{tb}")
                nc.scalar.activation(
                    stage[:], psum[:, :TQ],
                    mybir.ActivationFunctionType.Identity,
                    bias=biasT[:, o_abs // P:o_abs // P + 1],
                    scale=1.0)
                nc.scalar.dma_start(
                    out=yT_d[o_abs:o_abs + P, tb * TQ:(tb + 1) * TQ],
                    in_=stage[:])

            rep_ctx = tc.For_i(0, reps, 1) if reps > 1 else contextlib.nullcontext()
            with rep_ctx:
              # x-frontloaded prologue: quarter 0 + chunk 0 first (earliest
              # possible PE start), then the remaining quarters at full DMA
              # bandwidth while chunk 0's cells run, then W chunks stream
              # back-to-back, each unlocking more PE work than its DMA time.
              st0 = load_quarter_hi(0)
              wq0 = [load_chunk_half(0, 0)]
              load_quarter_lo(0, st0)
              wq0 += [load_chunk_half(0, h) for h in range(1, O_CHUNK // P)]
              for tb in range(NQ):
                  if tb > 0:
                      st = load_quarter_hi(tb)
                      load_quarter_lo(tb, st)
                  for ob in range(O_CHUNK // P):
                      cell(wq0[ob], 0, ob, tb)
              for ch in range(1, N_CHUNKS):
                  wqh = [load_chunk_half(ch, h) for h in range(O_CHUNK // P)]
                  for tb in range(NQ):
                      for ob in range(O_CHUNK // P):
                          cell(wqh[ob], ch, ob, tb)

    nc.compile()
    return nc


def kernel(input, weight, bias):
    global _built
    if _built is None:
        _built = _build()
    nc = _built
    from concourse.bass_utils import run_bass_kernel_spmd

    input = np.ascontiguousarray(input, dtype=np.float32)
    weight = np.ascontiguousarray(weight, dtype=np.float32)
    bias = np.ascontiguousarray(bias, dtype=np.float32)

    wT = np.ascontiguousarray(weight.T)                          # [i, o]
    biasT = np.ascontiguousarray(bias.reshape(OUT_F // P, P).T)  # [128, 32]

    in_maps = []
    for c in range(N_CORES):
        x_c = input[c * T_CORE:(c + 1) * T_CORE]                 # [1024, 4096]
        xT_c = np.ascontiguousarray(x_c.T)                       # [4096, 1024]
        in_maps.append({"xT": xT_c, "wT": wT, "biasT": biasT})

    res = run_bass_kernel_spmd(nc, in_maps, list(range(N_CORES)))

    y = np.empty((TOKENS, OUT_F), dtype=np.float32)
    for c in range(N_CORES):
        y[c * T_CORE:(c + 1) * T_CORE] = \
            np.asarray(res.results[c]["yT"]).astype(np.float32).T
    return y
